# revision 36
# baseline (speedup 1.0000x reference)
"""Trainium2 Bass kernel for nn_AttentionBlock (B=8, LN=2048, IDM=HDM=ODM=1024).

Sharding: data-parallel over batch, one batch element per NeuronCore (8 cores).

Algebra: scores = (i@q)(i@k)^T = i @ W @ i^T with W = q@k^T precomputed on
host in fp32. Per-core computation (batch element b):
    u       = i @ W                 [ln, idm]   (fp32r matmul)
    scores  = u @ i^T               [ln, ln]    (fp32r matmul)
    att     = softmax(scores, -1)
    vls     = i @ v                 [ln, idm]   (fp32r matmul)
    ret     = att @ vls + i                     (bf16 matmul)
    out     = leaky_relu(ret @ mlp, 0.2) + bias (bf16 matmul)

Precision strategy: the softmax exponent amplifies matmul operand rounding,
so the Q/K path (u, scores) runs in float32r — fp32 operands processed by
the PE at full bf16 rate (~13.5 effective mantissa bits, measured rms score
error ~1e-2 absolute vs std 82), no hi/lo splits needed. The value/MLP path
tolerates bf16. All accumulation is fp32 in PSUM.

Layout: contraction dim on partitions everywhere. iT (= i_b.T, fp32) stays
resident in SBUF; uT is staged through DRAM per 512-row s-group; att is
transposed on the fly by DMA.
"""
import numpy as np
import ml_dtypes

import concourse.bacc as bacc
import concourse.mybir as mybir
import concourse.tile as tile
from concourse import bass_utils

F32 = mybir.dt.float32
F32R = mybir.dt.float32r
BF16 = mybir.dt.bfloat16
I32 = mybir.dt.int32
Act = mybir.ActivationFunctionType
Axis = mybir.AxisListType

LN = 2048      # sequence length
D = 1024       # idm = hdm = odm
N_CORES = 8
DC = D // 128      # 8 contraction chunks
ST = LN // 128     # 16 s-tiles
TB = LN // 512     # 4 t-blocks (N=512)
G = LN // 512      # 4 s-groups
NEG_SLOPE = 0.2

_cached_nc = None


def _build(dbg=False):
    nc = bacc.Bacc("TRN2", target_bir_lowering=False, debug=False)

    iT = nc.dram_tensor("iT", [D, LN], F32R, kind="ExternalInput")
    w = nc.dram_tensor("w", [D, D], F32R, kind="ExternalInput")      # q @ k.T
    v = nc.dram_tensor("v", [D, D], F32R, kind="ExternalInput")
    mlpb = nc.dram_tensor("mlpb", [D, D], BF16, kind="ExternalInput")
    bias = nc.dram_tensor("bias", [LN, D], BF16, kind="ExternalInput")
    out_d = nc.dram_tensor("out", [LN, D], F32, kind="ExternalOutput")
    if dbg == 2:
        ud = nc.dram_tensor("ud", [G, 128, DC, 512], F32, kind="ExternalOutput")
        aTd = nc.dram_tensor("aTd", [G, 128, ST, 512], BF16, kind="ExternalOutput")
        attd2 = nc.dram_tensor("attd2", [ST, 128, LN], BF16, kind="ExternalOutput")
        retd = nc.dram_tensor("retd", [G, 128, DC, 512], BF16, kind="ExternalOutput")
    if dbg == 1:
        attd = nc.dram_tensor("attd", [ST, 128, LN], BF16, kind="ExternalOutput")
        attTd = nc.dram_tensor("attTd", [G, 128, ST, 512], BF16, kind="ExternalOutput")
        attTd2 = nc.dram_tensor("attTd2", [G, 128, ST, 512], BF16, kind="ExternalOutput")
        statsd = nc.dram_tensor("statsd", [ST, 128, 24], F32, kind="ExternalOutput")

    # [D, X] viewed as [128 partitions, DC chunks, X]
    def pcv(t, x):
        return t.ap().rearrange("(c p) x -> p c x", p=128)

    with tile.TileContext(nc) as tc:
        with tc.tile_pool(name="pers", bufs=1) as pers, \
             tc.tile_pool(name="dram", bufs=1, space="DRAM") as dram:
            iT_sb = pers.tile([128, DC, LN], F32R)     # 64 KB/part, resident
            vls_sb = pers.tile([128, ST, D], BF16)     # 32 KB/part, resident
            mlp_sb = pers.tile([128, DC, D], BF16)     # 16 KB/part, resident
            alpha_ap = pers.tile([128, 1], F32)
            nc.vector.memset(alpha_ap, NEG_SLOPE)

            uT_d = dram.tile([G, 128, DC, 512], F32R)   # staged u^T hi (r11)
            uTl_d = dram.tile([G, 128, DC, 512], F32R)  # staged u^T lo residual

            _psum_cm = tc.tile_pool(name="psum", bufs=1, space="PSUM")
            psum_pool = _psum_cm.__enter__()

            _pbu_cm = tc.tile_pool(name="pb_u", bufs=1)
            pb_u = _pbu_cm.__enter__()

            def acquire_u_tiles():
                t = pb_u.tile([128, DC, 512], F32R, name="uT_g", tag="uT_g")
                tl = pb_u.tile([128, DC, 512], F32R, name="uTl_g", tag="uTl_g")
                return t, tl

            u_tiles = {}

            # ================= Phase A: vls and uT =================
            with tc.tile_pool(name="pa_w", bufs=1) as pa_w, \
                 tc.tile_pool(name="pa_st", bufs=1) as pa_st:
                # v and W time-share one 32 KB buffer (pool generations)
                v_sb = pa_w.tile([128, DC, D], F32R, name="w_sb", tag="w")
                # interleave per-dc chunks so dc=0 deps resolve early
                for dc in range(DC):
                    nc.sync.dma_start(out=v_sb[:, dc], in_=pcv(v, D)[:, dc])
                    nc.sync.dma_start(out=iT_sb[:, dc], in_=pcv(iT, LN)[:, dc])
                nc.sync.dma_start(out=mlp_sb, in_=pcv(mlpb, D))

                # --- vls[t, e] = sum_d iT[d,t] v[d,e] -> resident bf16 ---
                for tc_ in range(ST):
                    t_sl = slice(tc_ * 128, tc_ * 128 + 128)
                    for eb in range(2):
                        ps = psum_pool.tile([128, 512], F32, name=f"pv{tc_}_{eb}",
                                            tag=f"av{(tc_ * 2 + eb) % 4}")
                        e_sl = slice(eb * 512, eb * 512 + 512)
                        for dc in range(DC):
                            nc.tensor.matmul(
                                ps,
                                iT_sb[:, dc, t_sl],
                                v_sb[:, dc, e_sl],
                                start=(dc == 0), stop=(dc == DC - 1),
                            )
                        nc.vector.tensor_copy(vls_sb[:, tc_, e_sl], ps)

                # --- uT[e, s] = sum_d W[d,e] iT[d,s] -> DRAM per s-group ---
                w_sb = pa_w.tile([128, DC, D], F32R, name="w_sb", tag="w")
                for dc in range(DC):
                    nc.sync.dma_start(out=w_sb[:, dc], in_=pcv(w, D)[:, dc])
                for g in range(G):
                    s_sl = slice(g * 512, g * 512 + 512)
                    for ec in range(DC):
                        ps = psum_pool.tile([128, 512], F32, name=f"pu{g}_{ec}",
                                            tag=f"sc{ec % 4}")
                        e_sl = slice(ec * 128, ec * 128 + 128)
                        for dc in range(DC):
                            nc.tensor.matmul(
                                ps,
                                w_sb[:, dc, e_sl],
                                iT_sb[:, dc, s_sl],
                                start=(dc == 0), stop=(dc == DC - 1),
                            )
                        # Split u into fp32r hi (r11 round-to-nearest, the
                        # grid both the DMA write-rounding and the PE operand
                        # read use) + residual lo, for a 2-pass hi/lo scores
                        # correction. Group 0 is written straight into the
                        # resident tiles by DVE; groups 1-3 stage via DRAM.
                        ust = pa_st.tile([128, 512], F32, name="ust", tag="ust")
                        nc.vector.tensor_copy(ust, ps)
                        uhst = pa_st.tile([128, 512], F32, name="uhst", tag="uhst")
                        nc.vector.tensor_scalar(
                            out=uhst.bitcast(I32), in0=ust.bitcast(I32),
                            scalar1=0x800, scalar2=None,
                            op0=mybir.AluOpType.add,
                        )
                        nc.vector.tensor_scalar(
                            out=uhst.bitcast(I32), in0=uhst.bitcast(I32),
                            scalar1=-4096, scalar2=None,
                            op0=mybir.AluOpType.bitwise_and,
                        )
                        nc.vector.tensor_sub(ust, ust, uhst)
                        nc.gpsimd.dma_start(out=uT_d[g, :, ec, :],
                                            in_=uhst.bitcast(F32R))
                        nc.gpsimd.dma_start(out=uTl_d[g, :, ec, :],
                                            in_=ust.bitcast(F32R))
                    if g == 0:
                        # prefetch group 0's u right behind its staging writes
                        u_tiles[0] = acquire_u_tiles()
                        nc.gpsimd.dma_start(out=u_tiles[0][0], in_=uT_d[0])
                        nc.gpsimd.dma_start(out=u_tiles[0][1], in_=uTl_d[0])

            # ================= Phase B: attention + MLP =================
            with tc.tile_pool(name="pb_nul", bufs=1) as _pb_nul, \
                 tc.tile_pool(name="pb_att", bufs=1) as pb_att, \
                 tc.tile_pool(name="pb_exp", bufs=2) as pb_exp, \
                 tc.tile_pool(name="pb_ret", bufs=2) as pb_ret, \
                 tc.tile_pool(name="pb_st", bufs=2) as pb_st, \
                 tc.tile_pool(name="pb_io", bufs=2) as pb_io:
                for g in range(G):
                    gs = slice(g * 512, g * 512 + 512)
                    uT_g, uTl_g = u_tiles[g]
                    if dbg == 2:
                        nc.gpsimd.dma_start(out=ud.ap()[g], in_=uT_g.bitcast(F32))
                    attT = pb_att.tile([128, ST, 512], BF16, name="attT", tag="attT")
                    ret_t = pb_ret.tile([128, DC, 512], BF16, name="ret", tag="ret")

                    for st4 in range(4):
                        si = g * 4 + st4
                        u_sl = slice(st4 * 128, st4 * 128 + 128)

                        scs = [
                            psum_pool.tile([128, 512], F32, name=f"sc{si}_{tb}",
                                           tag=f"sc{tb}")
                            for tb in range(TB)
                        ]
                        for ec in range(DC):
                            first = ec == 0
                            last = ec == DC - 1
                            lhs_h = uT_g[:, ec, u_sl]
                            lhs_l = uTl_g[:, ec, u_sl]
                            for tb in range(TB):
                                t_sl = slice(tb * 512, tb * 512 + 512)
                                nc.tensor.matmul(
                                    scs[tb], lhs_h,
                                    iT_sb[:, ec, t_sl],
                                    start=first, stop=False,
                                )
                                nc.tensor.matmul(
                                    scs[tb], lhs_l,
                                    iT_sb[:, ec, t_sl],
                                    start=False, stop=last,
                                )

                        # Per-t-block softmax: local max + exp immediately
                        # (frees each PSUM bank early), then algebraic
                        # rescale by f_tb = e^(m_tb - M) / S.
                        st_t = pb_st.tile([128, 24], F32, name="st_t", tag="stats")
                        negm4 = st_t[:, 0:4]
                        sums = st_t[:, 4:8]
                        negM = st_t[:, 8:9]
                        S = st_t[:, 9:10]
                        recip = st_t[:, 10:11]
                        g4 = st_t[:, 12:16]
                        f4 = st_t[:, 16:20]
                        gs4 = st_t[:, 20:24]
                        exp_t = pb_exp.tile([128, LN], F32, name="exp_t", tag="exp",
                                            bufs=1)
                        for tb in range(TB):
                            nc.vector.reduce_max(negm4[:, tb:tb + 1], scs[tb],
                                                 axis=Axis.X, negate=True)
                            nc.scalar.activation(
                                out=exp_t[:, tb * 512:tb * 512 + 512], in_=scs[tb],
                                func=Act.Exp, bias=negm4[:, tb:tb + 1], scale=1.0,
                                accum_out=sums[:, tb:tb + 1],
                            )
                        nc.vector.tensor_reduce(negM, negm4, axis=Axis.X,
                                                op=mybir.AluOpType.min)
                        nc.scalar.activation(out=g4, in_=negm4, func=Act.Exp,
                                             bias=negM, scale=-1.0)
                        nc.vector.tensor_mul(gs4, g4, sums)
                        nc.vector.reduce_sum(S, gs4, axis=Axis.X)
                        nc.vector.reciprocal(recip, S)
                        nc.vector.tensor_scalar_mul(f4, g4, recip)

                        att_t = pb_exp.tile([128, LN], BF16, name="att_t", tag="att")
                        for tb in range(TB):
                            nc.vector.tensor_scalar_mul(
                                att_t[:, tb * 512:tb * 512 + 512],
                                exp_t[:, tb * 512:tb * 512 + 512],
                                f4[:, tb:tb + 1],
                            )
                        # DVE 32x32 block transpose (avoids the xbar DMA
                        # transpose, which corrupts under concurrent DMA)
                        for pb in range(4):
                            for bi in range(4):
                                nc.vector.transpose(
                                    attT[pb * 32:(pb + 1) * 32, :,
                                         st4 * 128 + bi * 32:
                                         st4 * 128 + bi * 32 + 32],
                                    att_t[bi * 32:(bi + 1) * 32, :].rearrange(
                                        "p (t c x) -> p t c x", c=4, x=32
                                    )[:, :, pb, :],
                                )
                        if dbg == 2:
                            nc.gpsimd.dma_start(out=attd2.ap()[si], in_=att_t)
                        if dbg == 1:
                            nc.sync.dma_start(out=attd.ap()[si], in_=att_t)
                            nc.sync.dma_start(out=statsd.ap()[si], in_=st_t)

                    if dbg == 1:
                        nc.sync.dma_start(out=attTd.ap()[g], in_=attT)
                    if dbg == 2:
                        nc.gpsimd.dma_start(out=aTd.ap()[g], in_=attT)
                    # att @ vls (+ residual i) -> retT[e, s-block], bf16
                    for ec in range(DC):
                        psa = psum_pool.tile([128, 512], F32, name=f"pa{g}_{ec}",
                                             tag=f"av{ec % 4}")
                        e_sl = slice(ec * 128, ec * 128 + 128)
                        for tc_ in range(ST):
                            nc.tensor.matmul(
                                psa,
                                vls_sb[:, tc_, e_sl],
                                attT[:, tc_, :],
                                start=(tc_ == 0), stop=(tc_ == ST - 1),
                            )
                        nc.vector.tensor_add(ret_t[:, ec, :], psa,
                                             iT_sb[:, ec, gs].bitcast(F32))

                    if dbg == 2:
                        nc.gpsimd.dma_start(out=retd.ap()[g], in_=ret_t)
                    # (ret @ mlp) -> leaky relu -> + bias -> out
                    for st4 in range(4):
                        si = g * 4 + st4
                        s_sl = slice(si * 128, si * 128 + 128)
                        bias_t = pb_io.tile([128, D], BF16, name="bias_t", tag="bias")
                        nc.gpsimd.dma_start(out=bias_t, in_=bias.ap()[s_sl, :])
                        out_t = pb_io.tile([128, D], F32, name="out_t", tag="out")
                        for ob in range(2):
                            pso = psum_pool.tile([128, 512], F32, name=f"po{si}_{ob}",
                                                 tag=f"sc{ob}")
                            o_sl = slice(ob * 512, ob * 512 + 512)
                            for ec in range(DC):
                                nc.tensor.matmul(
                                    pso,
                                    ret_t[:, ec, st4 * 128:st4 * 128 + 128],
                                    mlp_sb[:, ec, o_sl],
                                    start=(ec == 0), stop=(ec == DC - 1),
                                )
                            nc.scalar.activation(
                                out=out_t[:, o_sl], in_=pso, func=Act.Prelu,
                                bias=0.0, scale=1.0, alpha=alpha_ap,
                            )
                        nc.vector.tensor_add(out_t, out_t, bias_t)
                        nc.gpsimd.dma_start(out=out_d.ap()[s_sl, :], in_=out_t)
                    if g + 1 < G:
                        u_tiles[g + 1] = acquire_u_tiles()
                        nc.gpsimd.dma_start(out=u_tiles[g + 1][0], in_=uT_d[g + 1])
                        nc.gpsimd.dma_start(out=u_tiles[g + 1][1], in_=uTl_d[g + 1])
                    if dbg == 1:
                        nc.sync.dma_start(out=attTd2.ap()[g], in_=attT)

            _pbu_cm.__exit__(None, None, None)
            _psum_cm.__exit__(None, None, None)

    nc.compile()
    return nc


def _get_nc():
    global _cached_nc
    if _cached_nc is None:
        _cached_nc = _build()
    return _cached_nc


def _prep_in_maps(i, k, q, v, mlp, bias):
    i = np.asarray(i, dtype=np.float32)
    k = np.asarray(k, dtype=np.float32)
    q = np.asarray(q, dtype=np.float32)
    v = np.asarray(v, dtype=np.float32)
    mlp = np.asarray(mlp, dtype=np.float32)
    bias = np.asarray(bias, dtype=np.float32)

    w = (q.astype(np.float64) @ k.astype(np.float64).T).astype(np.float32)
    shared = dict(
        w=w, v=v,
        mlpb=mlp.astype(ml_dtypes.bfloat16),
        bias=bias.astype(ml_dtypes.bfloat16),
    )
    in_maps = []
    for b in range(N_CORES):
        iT = np.ascontiguousarray(i[b].T)
        in_maps.append(dict(iT=iT, **shared))
    return in_maps


def kernel(i, k, q, v, mlp, bias):
    in_maps = _prep_in_maps(i, k, q, v, mlp, bias)
    nc = _get_nc()
    res = bass_utils.run_bass_kernel_spmd(nc, in_maps, core_ids=list(range(N_CORES)))
    return np.stack([res.results[b]["out"] for b in range(N_CORES)])
